# revision 9
# baseline (speedup 1.0000x reference)
"""Cost-volume kernel for Trainium2 (Bass/Tile), 8-core SPMD.

volume[n, c, d, h, w] = left[n,c,h,w] * right[n,c,h,w-d]  (0 where w < d)

Sharding: rows (flattened n,c,h = 8704) split as 1088 per core; every core
computes ALL 48 disparities for its rows (shift is along W, so row sharding
needs no halo and inputs are read once).

The kernel is HBM-store bound, so the store stream is minimized two ways:
 - fp16 output (harness gate is rel_err < 2e-2; fp16 product error ~7e-4).
 - packed layout: for disparity d only the ~(W-d) valid products
   packed[d][r, j] = left[r, d+j] * right[r, j] are stored; the host
   scatters them into a zero-filled full volume.

All multiplies run on DVE (gpsimd tensor_tensor measured ~3x slower and the
ACT engine has no two-tensor op). DVE does ~0.52 ns/elem (2x_1p fp16 mode)
plus ~350 ns fixed cost per instruction, so disparities are processed in
GROUPS of 4 per instruction using a 4-D access pattern whose group dim has
stride +1 on the left operand (one extra shift per group member) and
stride 0 (broadcast) on the right operand. Group blocks share a uniform
width W-g, so members i>0 carry (d-g) junk columns that the host ignores.

Main chunk: rows [64,1088) as [128 partitions x 8 rows]; per-partition
lines are DRAM-contiguous so every load/store is a >=3 KB-per-partition
single DMA. Disparities 0..3 are emitted per-d (even width) so the store
stream starts after ~1.4 us; d 4..47 go in 11 groups of 4. The 64-row
tail is ONE flat [64, 48, 240] multiply + one store. Big stores ride the
ACT HWDGE ring; loads and the tail store ride SP.
"""

import os

import numpy as np

import bass_rust
import concourse.bacc as bacc
import concourse.mybir as mybir
from concourse.bass_utils import run_bass_kernel_spmd
from concourse.mybir import AluOpType
from concourse.tile import TileContext

N, C, H, W = 2, 32, 136, 240
MAX_DISP = 48
NCORES = 8
R = N * C * H                   # 8704 rows total
ROWS = R // NCORES              # 1088 rows per core
TAIL = 64                       # leftover rows (1088 = 64 + 128*8)
CPP = 8                         # rows per partition in the main chunk
G = 4                           # disparities per grouped DVE instruction
NSOLO = 4                       # leading disparities emitted per-d
LBW = CPP * W + 8               # lb tile width (pad: group reads to 1919+3)
LTW = W + MAX_DISP              # lt tile width (tail reads to 286)

# Even-rounded block width for the per-d leading blocks (alignment-safe).
BW = [W - d + ((W - d) & 1) for d in range(NSOLO)]

# out_big per-partition column offsets: NSOLO per-d blocks [8, BW[d]] then
# 11 groups [G, 8, W-g].
XB = {}
_col = 0
for _d in range(NSOLO):
    XB[_d] = _col
    _col += CPP * BW[_d]
for _g in range(NSOLO, MAX_DISP, G):
    XB[_g] = _col
    _col += G * CPP * (W - _g)
XBTOT = _col                    # 83680

_NC_CACHE = None
LAST_RESULTS = None  # BassKernelResults of the most recent run (for test.py)


def _build_bass():
    # Bacc (not plain Bass): its finalize() runs the compile pipeline incl.
    # generate_event_semaphores, which splits multi-sem waits that walrus
    # rejects ("Too many sync wait commands").
    nc = bacc.Bacc()
    left = nc.dram_tensor("left", [ROWS, W], mybir.dt.float16, kind="ExternalInput")
    right = nc.dram_tensor("right", [ROWS, W], mybir.dt.float16, kind="ExternalInput")
    out_big = nc.dram_tensor(
        "out_big", [128, XBTOT], mybir.dt.float16, kind="ExternalOutput"
    )
    out_tail = nc.dram_tensor(
        "out_tail", [TAIL, MAX_DISP * W], mybir.dt.float16, kind="ExternalOutput"
    )

    with (
        TileContext(nc) as tc,
        tc.tile_pool(name="lpool", bufs=1) as lpool,
        tc.tile_pool(name="rpool", bufs=1) as rpool,
        tc.tile_pool(name="osolo", bufs=4) as osolo,
        tc.tile_pool(name="ogrp", bufs=3) as ogrp,
        tc.tile_pool(name="otail", bufs=1) as otail,
    ):
        lb = lpool.tile([128, LBW], mybir.dt.float16, tag="lbig")
        rb = rpool.tile([128, CPP * W], mybir.dt.float16, tag="rbig")
        lt = lpool.tile([TAIL, LTW], mybir.dt.float16, tag="ltail")
        rt = rpool.tile([TAIL, W], mybir.dt.float16, tag="rtail")

        # Loads split across both HWDGE rings so trigger issue (~0.7 us
        # each) overlaps and data lands sooner.
        nc.sync.dma_start(
            out=lb[:, 0 : CPP * W],
            in_=left[TAIL:ROWS, :].rearrange("(p q) w -> p (q w)", p=128),
        )
        nc.scalar.dma_start(
            out=rb[:],
            in_=right[TAIL:ROWS, :].rearrange("(p q) w -> p (q w)", p=128),
        )
        nc.sync.dma_start(out=lt[:, 0:W], in_=left[0:TAIL, :])
        nc.scalar.dma_start(out=rt[:], in_=right[0:TAIL, :])

        lbv = lb[:, 0 : CPP * W].rearrange("p (q w) -> p q w", w=W)
        rbv = rb[:].rearrange("p (q w) -> p q w", w=W)
        lb_ap = lb[:]

        # Leading disparities per-d: store stream starts after one ~1 us op.
        for d in range(NSOLO):
            bw = BW[d]
            ob = osolo.tile([128, CPP * W], mybir.dt.float16)
            # d=0 goes in two q-halves so the first store trigger fires one
            # half-multiply (~0.7 us) sooner.
            for q0, q1 in ([(0, 4), (4, CPP)] if d == 0 else [(0, CPP)]):
                nq = q1 - q0
                in0 = bass_rust.AP(
                    lb_ap.tensor,
                    lb_ap.offset + q0 * W + d,
                    [[LBW, 128], [W, nq], [1, bw]],
                )
                nc.vector.tensor_tensor(
                    ob[:, q0 * bw : q1 * bw].rearrange("p (q w) -> p q w", w=bw),
                    in0,
                    rbv[:, q0:q1, 0:bw],
                    AluOpType.mult,
                )
                nc.scalar.dma_start(
                    out=out_big[:, XB[d] + q0 * bw : XB[d] + q1 * bw],
                    in_=ob[:, q0 * bw : q1 * bw],
                )

        # Grouped disparities: one 4-D instruction per 4 d's. Left operand
        # group dim strides +1 (shift), right operand broadcasts.
        for g in range(NSOLO, MAX_DISP, G):
            wg = W - g
            ob = ogrp.tile([128, G * CPP * (W - NSOLO)], mybir.dt.float16)
            in0 = bass_rust.AP(
                lb_ap.tensor,
                lb_ap.offset + g,
                [[LBW, 128], [1, G], [W, CPP], [1, wg]],
            )
            in1 = rbv[:, :, 0:wg].unsqueeze(1).broadcast_to([128, G, CPP, wg])
            nc.vector.tensor_tensor(
                ob[:, 0 : G * CPP * wg].rearrange(
                    "p (i q w) -> p i q w", i=G, q=CPP
                ),
                in0,
                in1,
                AluOpType.mult,
            )
            nc.scalar.dma_start(
                out=out_big[:, XB[g] : XB[g] + G * CPP * wg],
                in_=ob[:, 0 : G * CPP * wg],
            )
            if g == NSOLO:
                # Tail: one flat [64, 48, 240] multiply on the otherwise-idle
                # Pool engine (runs concurrently with the whole DVE stream),
                # one store on the SP ring.
                ot = otail.tile([TAIL, MAX_DISP * W], mybir.dt.float16)
                t_in0 = bass_rust.AP(
                    lt[:].tensor,
                    lt[:].offset,
                    [[LTW, TAIL], [1, MAX_DISP], [1, W]],
                )
                t_in1 = rt[:].unsqueeze(1).broadcast_to([TAIL, MAX_DISP, W])
                nc.gpsimd.tensor_tensor(
                    ot[:].rearrange("p (i w) -> p i w", w=W),
                    t_in0,
                    t_in1,
                    AluOpType.mult,
                )
                nc.sync.dma_start(out=out_tail[:, :], in_=ot[:])
    nc.finalize()
    return nc


def kernel(left: np.ndarray, right: np.ndarray) -> np.ndarray:
    global _NC_CACHE, LAST_RESULTS
    left = np.ascontiguousarray(np.asarray(left, dtype=np.float32))
    right = np.ascontiguousarray(np.asarray(right, dtype=np.float32))
    assert left.shape == (N, C, H, W) and right.shape == (N, C, H, W)

    if _NC_CACHE is None:
        _NC_CACHE = _build_bass()
    nc = _NC_CACHE

    left_flat = np.ascontiguousarray(left.reshape(R, W).astype(np.float16))
    right_flat = np.ascontiguousarray(right.reshape(R, W).astype(np.float16))
    in_maps = [
        {
            "left": left_flat[ROWS * k : ROWS * (k + 1)],
            "right": right_flat[ROWS * k : ROWS * (k + 1)],
        }
        for k in range(NCORES)
    ]

    trace = os.environ.get("COSTVOL_TRACE", "0") == "1"
    kwargs = {}
    if os.environ.get("COSTVOL_TRACE_ALL", "0") == "1":
        kwargs["trace_cores"] = list(range(NCORES))
    res = run_bass_kernel_spmd(
        nc, in_maps, list(range(NCORES)), trace=trace, **kwargs
    )
    LAST_RESULTS = res

    flat = np.zeros((MAX_DISP, R, W), dtype=np.float32)
    for k in range(NCORES):
        big = np.asarray(res.results[k]["out_big"])
        tail = np.asarray(res.results[k]["out_tail"]).reshape(TAIL, MAX_DISP, W)
        r0 = ROWS * k
        for d in range(NSOLO):
            w = W - d
            blk = big[:, XB[d] : XB[d] + CPP * BW[d]].reshape(128, CPP, BW[d])
            flat[d, r0 + TAIL : r0 + ROWS, d:] = (
                blk[:, :, :w].astype(np.float32).reshape(128 * CPP, w)
            )
        for g in range(NSOLO, MAX_DISP, G):
            wg = W - g
            blk = big[:, XB[g] : XB[g] + G * CPP * wg].reshape(128, G, CPP, wg)
            for i in range(G):
                d = g + i
                w = W - d
                flat[d, r0 + TAIL : r0 + ROWS, d:] = (
                    blk[:, i, :, :w].astype(np.float32).reshape(128 * CPP, w)
                )
        for d in range(MAX_DISP):
            flat[d, r0 : r0 + TAIL, d:] = tail[:, d, : W - d].astype(np.float32)
    vol = flat.reshape(MAX_DISP, N, C, H, W).transpose(1, 2, 0, 3, 4)
    return np.ascontiguousarray(vol)


# revision 10
# speedup vs baseline: 1.0991x; 1.0991x over previous
"""Cost-volume kernel for Trainium2 (Bass/Tile), 8-core SPMD.

volume[n, c, d, h, w] = left[n,c,h,w] * right[n,c,h,w-d]  (0 where w < d)

Sharding: rows (flattened n,c,h = 8704) split as 1088 per core; every core
computes ALL 48 disparities for its rows (shift is along W, so row sharding
needs no halo and inputs are read once).

The kernel is HBM-store bound, so the store stream is minimized two ways:
 - fp16 output (harness gate is rel_err < 2e-2; fp16 product error ~7e-4).
 - packed layout: for disparity d only the ~(W-d) valid products
   packed[d][r, j] = left[r, d+j] * right[r, j] are stored; the host
   scatters them into a zero-filled full volume.

All multiplies run on DVE (gpsimd tensor_tensor measured ~3x slower and the
ACT engine has no two-tensor op). DVE does ~0.52 ns/elem (2x_1p fp16 mode)
plus ~350 ns fixed cost per instruction, so disparities are processed in
GROUPS of 4 per instruction using a 4-D access pattern whose group dim has
stride +1 on the left operand (one extra shift per group member) and
stride 0 (broadcast) on the right operand. Group blocks share a uniform
width W-g, so members i>0 carry (d-g) junk columns that the host ignores.

Main chunk: rows [64,1088) as [128 partitions x 8 rows]; per-partition
lines are DRAM-contiguous so every load/store is a >=3 KB-per-partition
single DMA. Disparities 0..3 are emitted per-d (even width) so the store
stream starts after ~1.4 us; d 4..47 go in 11 groups of 4. The 64-row
tail is ONE flat [64, 48, 240] multiply + one store. Big stores ride the
ACT HWDGE ring; loads and the tail store ride SP.
"""

import os

import numpy as np

import bass_rust
import concourse.bacc as bacc
import concourse.mybir as mybir
from concourse.bass_utils import run_bass_kernel_spmd
from concourse.mybir import AluOpType
from concourse.tile import TileContext

N, C, H, W = 2, 32, 136, 240
MAX_DISP = 48
NCORES = 8
R = N * C * H                   # 8704 rows total
ROWS = R // NCORES              # 1088 rows per core
TAIL = 64                       # leftover rows (1088 = 64 + 128*8)
CPP = 8                         # rows per partition in the main chunk
G = 4                           # disparities per grouped DVE instruction
NSOLO = 4                       # leading disparities emitted per-d
LBW = CPP * W + 8               # lb tile width (pad: group reads to 1919+3)
LTW = W + MAX_DISP              # lt tile width (tail reads to 286)

# Even-rounded block width for the per-d leading blocks (alignment-safe).
BW = [W - d + ((W - d) & 1) for d in range(NSOLO)]

# out_big per-partition column offsets: NSOLO per-d blocks [8, BW[d]] then
# 11 groups [G, 8, W-g].
XB = {}
_col = 0
for _d in range(NSOLO):
    XB[_d] = _col
    _col += CPP * BW[_d]
for _g in range(NSOLO, MAX_DISP, G):
    XB[_g] = _col
    _col += G * CPP * (W - _g)
XBTOT = _col                    # 83680

_NC_CACHE = None
LAST_RESULTS = None  # BassKernelResults of the most recent run (for test.py)


def _build_bass():
    # Bacc (not plain Bass): its finalize() runs the compile pipeline incl.
    # generate_event_semaphores, which splits multi-sem waits that walrus
    # rejects ("Too many sync wait commands").
    nc = bacc.Bacc()
    left = nc.dram_tensor("left", [ROWS, W], mybir.dt.float16, kind="ExternalInput")
    right = nc.dram_tensor("right", [ROWS, W], mybir.dt.float16, kind="ExternalInput")
    out_big = nc.dram_tensor(
        "out_big", [128, XBTOT], mybir.dt.float16, kind="ExternalOutput"
    )
    out_tail = nc.dram_tensor(
        "out_tail", [TAIL, MAX_DISP * W], mybir.dt.float16, kind="ExternalOutput"
    )

    with (
        TileContext(nc) as tc,
        tc.tile_pool(name="lpool", bufs=1) as lpool,
        tc.tile_pool(name="rpool", bufs=1) as rpool,
        tc.tile_pool(name="osolo", bufs=4) as osolo,
        tc.tile_pool(name="ogrp", bufs=3) as ogrp,
        tc.tile_pool(name="otail", bufs=1) as otail,
    ):
        lb = lpool.tile([128, LBW], mybir.dt.float16, tag="lbig")
        rb = rpool.tile([128, CPP * W], mybir.dt.float16, tag="rbig")
        lt = lpool.tile([TAIL, LTW], mybir.dt.float16, tag="ltail")
        rt = rpool.tile([TAIL, W], mybir.dt.float16, tag="rtail")

        # Loads split across both HWDGE rings so trigger issue (~0.7 us
        # each) overlaps and data lands sooner.
        nc.sync.dma_start(
            out=lb[:, 0 : CPP * W],
            in_=left[TAIL:ROWS, :].rearrange("(p q) w -> p (q w)", p=128),
        )
        nc.scalar.dma_start(
            out=rb[:],
            in_=right[TAIL:ROWS, :].rearrange("(p q) w -> p (q w)", p=128),
        )
        nc.sync.dma_start(out=lt[:, 0:W], in_=left[0:TAIL, :])
        nc.scalar.dma_start(out=rt[:], in_=right[0:TAIL, :])

        lbv = lb[:, 0 : CPP * W].rearrange("p (q w) -> p q w", w=W)
        rbv = rb[:].rearrange("p (q w) -> p q w", w=W)
        lb_ap = lb[:]

        # Leading disparities per-d: store stream starts after one ~1 us op.
        for d in range(NSOLO):
            bw = BW[d]
            ob = osolo.tile([128, CPP * W], mybir.dt.float16)
            # d=0 goes in two q-halves so the first store trigger fires one
            # half-multiply (~0.7 us) sooner.
            for q0, q1 in ([(0, 4), (4, CPP)] if d == 0 else [(0, CPP)]):
                nq = q1 - q0
                in0 = bass_rust.AP(
                    lb_ap.tensor,
                    lb_ap.offset + q0 * W + d,
                    [[LBW, 128], [W, nq], [1, bw]],
                )
                nc.vector.tensor_tensor(
                    ob[:, q0 * bw : q1 * bw].rearrange("p (q w) -> p q w", w=bw),
                    in0,
                    rbv[:, q0:q1, 0:bw],
                    AluOpType.mult,
                )
                nc.scalar.dma_start(
                    out=out_big[:, XB[d] + q0 * bw : XB[d] + q1 * bw],
                    in_=ob[:, q0 * bw : q1 * bw],
                )

        # Grouped disparities: one 4-D instruction per 4 d's. Left operand
        # group dim strides +1 (shift), right operand broadcasts.
        for g in range(NSOLO, MAX_DISP, G):
            wg = W - g
            ob = ogrp.tile([128, G * CPP * (W - NSOLO)], mybir.dt.float16)
            in0 = bass_rust.AP(
                lb_ap.tensor,
                lb_ap.offset + g,
                [[LBW, 128], [1, G], [W, CPP], [1, wg]],
            )
            in1 = rbv[:, :, 0:wg].unsqueeze(1).broadcast_to([128, G, CPP, wg])
            nc.vector.tensor_tensor(
                ob[:, 0 : G * CPP * wg].rearrange(
                    "p (i q w) -> p i q w", i=G, q=CPP
                ),
                in0,
                in1,
                AluOpType.mult,
            )
            nc.scalar.dma_start(
                out=out_big[:, XB[g] : XB[g] + G * CPP * wg],
                in_=ob[:, 0 : G * CPP * wg],
            )
            if g == 2 * G + NSOLO:
                # Tail: one flat [64, 48, 240] multiply + one store. On DVE:
                # a concurrent Pool op stalls DVE for its whole duration
                # (SBUF contention), so Pool is useless here.
                ot = otail.tile([TAIL, MAX_DISP * W], mybir.dt.float16)
                t_in0 = bass_rust.AP(
                    lt[:].tensor,
                    lt[:].offset,
                    [[LTW, TAIL], [1, MAX_DISP], [1, W]],
                )
                t_in1 = rt[:].unsqueeze(1).broadcast_to([TAIL, MAX_DISP, W])
                nc.vector.tensor_tensor(
                    ot[:].rearrange("p (i w) -> p i w", w=W),
                    t_in0,
                    t_in1,
                    AluOpType.mult,
                )
                nc.sync.dma_start(out=out_tail[:, :], in_=ot[:])
    nc.finalize()
    return nc


def kernel(left: np.ndarray, right: np.ndarray) -> np.ndarray:
    global _NC_CACHE, LAST_RESULTS
    left = np.ascontiguousarray(np.asarray(left, dtype=np.float32))
    right = np.ascontiguousarray(np.asarray(right, dtype=np.float32))
    assert left.shape == (N, C, H, W) and right.shape == (N, C, H, W)

    if _NC_CACHE is None:
        _NC_CACHE = _build_bass()
    nc = _NC_CACHE

    left_flat = np.ascontiguousarray(left.reshape(R, W).astype(np.float16))
    right_flat = np.ascontiguousarray(right.reshape(R, W).astype(np.float16))
    in_maps = [
        {
            "left": left_flat[ROWS * k : ROWS * (k + 1)],
            "right": right_flat[ROWS * k : ROWS * (k + 1)],
        }
        for k in range(NCORES)
    ]

    trace = os.environ.get("COSTVOL_TRACE", "0") == "1"
    kwargs = {}
    if os.environ.get("COSTVOL_TRACE_ALL", "0") == "1":
        kwargs["trace_cores"] = list(range(NCORES))
    res = run_bass_kernel_spmd(
        nc, in_maps, list(range(NCORES)), trace=trace, **kwargs
    )
    LAST_RESULTS = res

    flat = np.zeros((MAX_DISP, R, W), dtype=np.float32)
    for k in range(NCORES):
        big = np.asarray(res.results[k]["out_big"])
        tail = np.asarray(res.results[k]["out_tail"]).reshape(TAIL, MAX_DISP, W)
        r0 = ROWS * k
        for d in range(NSOLO):
            w = W - d
            blk = big[:, XB[d] : XB[d] + CPP * BW[d]].reshape(128, CPP, BW[d])
            flat[d, r0 + TAIL : r0 + ROWS, d:] = (
                blk[:, :, :w].astype(np.float32).reshape(128 * CPP, w)
            )
        for g in range(NSOLO, MAX_DISP, G):
            wg = W - g
            blk = big[:, XB[g] : XB[g] + G * CPP * wg].reshape(128, G, CPP, wg)
            for i in range(G):
                d = g + i
                w = W - d
                flat[d, r0 + TAIL : r0 + ROWS, d:] = (
                    blk[:, i, :, :w].astype(np.float32).reshape(128 * CPP, w)
                )
        for d in range(MAX_DISP):
            flat[d, r0 : r0 + TAIL, d:] = tail[:, d, : W - d].astype(np.float32)
    vol = flat.reshape(MAX_DISP, N, C, H, W).transpose(1, 2, 0, 3, 4)
    return np.ascontiguousarray(vol)


# revision 18
# speedup vs baseline: 1.2123x; 1.1030x over previous
"""Cost-volume kernel for Trainium2 (Bass/Tile), 8-core SPMD.

volume[n, c, d, h, w] = left[n,c,h,w] * right[n,c,h,w-d]  (0 where w < d)

Sharding: rows (flattened n,c,h = 8704) split as 1088 per core; every core
computes ALL 48 disparities for its rows (shift is along W, so row sharding
needs no halo and inputs are read once).

The kernel is HBM-store bound, so the store stream is minimized two ways:
 - fp16 output (harness gate is rel_err < 2e-2; fp16 product error ~7e-4).
 - packed layout: for disparity d only the ~(W-d) valid products
   packed[d][r, j] = left[r, d+j] * right[r, j] are stored; the host
   scatters them into a zero-filled full volume.

All multiplies run on DVE (gpsimd tensor_tensor measured ~3x slower and the
ACT engine has no two-tensor op). DVE does ~0.52 ns/elem (2x_1p fp16 mode)
plus ~350 ns fixed cost per instruction, so disparities are processed in
GROUPS of 4 per instruction using a 4-D access pattern whose group dim has
stride +1 on the left operand (one extra shift per group member) and
stride 0 (broadcast) on the right operand. Group blocks share a uniform
width W-g, so members i>0 carry (d-g) junk columns that the host ignores.

Main chunk: rows [64,1088) as [128 partitions x 8 rows]; per-partition
lines are DRAM-contiguous so every load/store is a >=3 KB-per-partition
single DMA. Disparities 0..3 are emitted per-d (even width) so the store
stream starts after ~1.4 us; d 4..47 go in 11 groups of 4. The 64-row
tail is ONE flat [64, 48, 240] multiply + one store. Big stores ride the
ACT HWDGE ring; loads and the tail store ride SP.
"""

import os

import numpy as np

import bass_rust
import concourse.bacc as bacc
import concourse.mybir as mybir
from concourse.bass_utils import run_bass_kernel_spmd
from concourse.mybir import AluOpType
from concourse.tile import TileContext

N, C, H, W = 2, 32, 136, 240
MAX_DISP = 48
NCORES = 8
R = N * C * H                   # 8704 rows total
ROWS = R // NCORES              # 1088 rows per core
TAIL = 64                       # leftover rows (1088 = 64 + 128*8)
CPP = 8                         # rows per partition in the main chunk
G = 4                           # disparities per grouped DVE instruction
NSOLO = 4                       # leading disparities emitted per-d
LBW = CPP * W + 8               # lb tile width (pad: group reads to 1919+3)
LTW = W + MAX_DISP              # lt tile width (tail reads to 286)

# Per-d blocks: leading d 0..3 (pipeline ramp) and trailing d 44..47
# (drain taper). Groups of G=4 cover d 4..43.
SOLOS = list(range(NSOLO)) + list(range(MAX_DISP - NSOLO, MAX_DISP))
GROUPS = list(range(NSOLO, MAX_DISP - NSOLO, G))

# Even-rounded block width for the per-d blocks (alignment-safe).
BW = {d: W - d + ((W - d) & 1) for d in SOLOS}

# out_big per-partition column offsets.
XB = {}
_col = 0
for _d in SOLOS:
    XB[_d] = _col
    _col += CPP * BW[_d]
for _g in GROUPS:
    XB[_g] = _col
    _col += G * CPP * (W - _g)
XBTOT = _col

_NC_CACHE = None
LAST_RESULTS = None  # BassKernelResults of the most recent run (for test.py)


def _build_bass():
    # Bacc (not plain Bass): its finalize() runs the compile pipeline incl.
    # generate_event_semaphores, which splits multi-sem waits that walrus
    # rejects ("Too many sync wait commands").
    nc = bacc.Bacc()
    left = nc.dram_tensor("left", [ROWS, W], mybir.dt.float16, kind="ExternalInput")
    right = nc.dram_tensor("right", [ROWS, W], mybir.dt.float16, kind="ExternalInput")
    out_big = nc.dram_tensor(
        "out_big", [128, XBTOT], mybir.dt.float16, kind="ExternalOutput"
    )
    out_tail = nc.dram_tensor(
        "out_tail", [TAIL, MAX_DISP * W], mybir.dt.float16, kind="ExternalOutput"
    )

    with (
        TileContext(nc) as tc,
        tc.tile_pool(name="lpool", bufs=1) as lpool,
        tc.tile_pool(name="rpool", bufs=1) as rpool,
        tc.tile_pool(name="osolo", bufs=4) as osolo,
        tc.tile_pool(name="ogrp", bufs=5) as ogrp,
        tc.tile_pool(name="otail", bufs=1) as otail,
    ):
        lb = lpool.tile([128, LBW], mybir.dt.float16, tag="lbig")
        rb = rpool.tile([128, CPP * W], mybir.dt.float16, tag="rbig")
        lt = lpool.tile([TAIL, LTW], mybir.dt.float16, tag="ltail")
        rt = rpool.tile([TAIL, W], mybir.dt.float16, tag="rtail")

        # Loads split across both HWDGE rings and into q-halves, so the
        # d=0 first-half multiply (which only reads q 0..3) can start after
        # ~1.5 us of load data instead of the full 2.6 us.
        HQ = CPP // 2
        lsrc = left[TAIL:ROWS, :].rearrange("(p q) w -> p q w", p=128)
        rsrc = right[TAIL:ROWS, :].rearrange("(p q) w -> p q w", p=128)
        lbq = lb[:, 0 : CPP * W].rearrange("p (q w) -> p q w", w=W)
        rbq = rb[:].rearrange("p (q w) -> p q w", w=W)
        nc.sync.dma_start(out=lbq[:, 0:HQ, :], in_=lsrc[:, 0:HQ, :])
        nc.scalar.dma_start(out=rbq[:, 0:HQ, :], in_=rsrc[:, 0:HQ, :])
        nc.sync.dma_start(out=lbq[:, HQ:CPP, :], in_=lsrc[:, HQ:CPP, :])
        nc.scalar.dma_start(out=rbq[:, HQ:CPP, :], in_=rsrc[:, HQ:CPP, :])
        nc.sync.dma_start(out=lt[:, 0:W], in_=left[0:TAIL, :])
        nc.scalar.dma_start(out=rt[:], in_=right[0:TAIL, :])

        lbv = lb[:, 0 : CPP * W].rearrange("p (q w) -> p q w", w=W)
        rbv = rb[:].rearrange("p (q w) -> p q w", w=W)
        lb_ap = lb[:]

        # Leading disparities per-d: store stream starts after one ~1 us op.
        def solo(d):
            bw = BW[d]
            ob = osolo.tile([128, CPP * W], mybir.dt.float16)
            # d=0 goes in two q-halves so the first store trigger fires one
            # half-multiply (~0.7 us) sooner (and only waits on the first
            # half-loads).
            for q0, q1 in ([(0, HQ), (HQ, CPP)] if d == 0 else [(0, CPP)]):
                nq = q1 - q0
                in0 = bass_rust.AP(
                    lb_ap.tensor,
                    lb_ap.offset + q0 * W + d,
                    [[LBW, 128], [W, nq], [1, bw]],
                )
                nc.vector.tensor_tensor(
                    ob[:, q0 * bw : q1 * bw].rearrange("p (q w) -> p q w", w=bw),
                    in0,
                    rbv[:, q0:q1, 0:bw],
                    AluOpType.mult,
                )
                nc.scalar.dma_start(
                    out=out_big[:, XB[d] + q0 * bw : XB[d] + q1 * bw],
                    in_=ob[:, q0 * bw : q1 * bw],
                )

        for d in range(NSOLO):
            solo(d)

        # Grouped disparities: one 4-D instruction per 4 d's. Left operand
        # group dim strides +1 (shift), right operand broadcasts.
        for g in GROUPS:
            wg = W - g
            ob = ogrp.tile([128, G * CPP * (W - NSOLO)], mybir.dt.float16)
            in0 = bass_rust.AP(
                lb_ap.tensor,
                lb_ap.offset + g,
                [[LBW, 128], [1, G], [W, CPP], [1, wg]],
            )
            in1 = rbv[:, :, 0:wg].unsqueeze(1).broadcast_to([128, G, CPP, wg])
            nc.vector.tensor_tensor(
                ob[:, 0 : G * CPP * wg].rearrange(
                    "p (i q w) -> p i q w", i=G, q=CPP
                ),
                in0,
                in1,
                AluOpType.mult,
            )
            nc.scalar.dma_start(
                out=out_big[:, XB[g] : XB[g] + G * CPP * wg],
                in_=ob[:, 0 : G * CPP * wg],
            )
            if g == 2 * G + NSOLO:
                # Tail: one flat [64, 48, 240] multiply + one store. On DVE:
                # a concurrent Pool op stalls DVE for its whole duration
                # (SBUF contention), so Pool is useless here.
                ot = otail.tile([TAIL, MAX_DISP * W], mybir.dt.float16)
                t_in0 = bass_rust.AP(
                    lt[:].tensor,
                    lt[:].offset,
                    [[LTW, TAIL], [1, MAX_DISP], [1, W]],
                )
                t_in1 = rt[:].unsqueeze(1).broadcast_to([TAIL, MAX_DISP, W])
                nc.vector.tensor_tensor(
                    ot[:].rearrange("p (i w) -> p i w", w=W),
                    t_in0,
                    t_in1,
                    AluOpType.mult,
                )
                # Two half stores: one 23 KB-per-partition store runs at
                # half DMA-engine rate; <=16 KB packets run at full rate.
                half = MAX_DISP * W // 2
                nc.sync.dma_start(out=out_tail[:, 0:half], in_=ot[:, 0:half])
                nc.sync.dma_start(out=out_tail[:, half:], in_=ot[:, half:])

        # Drain taper: small per-d blocks at the end so the final store
        # backlog after the last multiply is ~0.4 MB, not ~1.6 MB.
        for d in range(MAX_DISP - NSOLO, MAX_DISP):
            solo(d)
    nc.finalize()
    return nc


def kernel(left: np.ndarray, right: np.ndarray) -> np.ndarray:
    global _NC_CACHE, LAST_RESULTS
    left = np.ascontiguousarray(np.asarray(left, dtype=np.float32))
    right = np.ascontiguousarray(np.asarray(right, dtype=np.float32))
    assert left.shape == (N, C, H, W) and right.shape == (N, C, H, W)

    if _NC_CACHE is None:
        _NC_CACHE = _build_bass()
    nc = _NC_CACHE

    left_flat = np.ascontiguousarray(left.reshape(R, W).astype(np.float16))
    right_flat = np.ascontiguousarray(right.reshape(R, W).astype(np.float16))
    in_maps = [
        {
            "left": left_flat[ROWS * k : ROWS * (k + 1)],
            "right": right_flat[ROWS * k : ROWS * (k + 1)],
        }
        for k in range(NCORES)
    ]

    trace = os.environ.get("COSTVOL_TRACE", "0") == "1"
    kwargs = {}
    if os.environ.get("COSTVOL_TRACE_ALL", "0") == "1":
        kwargs["trace_cores"] = list(range(NCORES))
    res = run_bass_kernel_spmd(
        nc, in_maps, list(range(NCORES)), trace=trace, **kwargs
    )
    LAST_RESULTS = res

    flat = np.zeros((MAX_DISP, R, W), dtype=np.float32)
    for k in range(NCORES):
        big = np.asarray(res.results[k]["out_big"])
        tail = np.asarray(res.results[k]["out_tail"]).reshape(TAIL, MAX_DISP, W)
        r0 = ROWS * k
        for d in SOLOS:
            w = W - d
            blk = big[:, XB[d] : XB[d] + CPP * BW[d]].reshape(128, CPP, BW[d])
            flat[d, r0 + TAIL : r0 + ROWS, d:] = (
                blk[:, :, :w].astype(np.float32).reshape(128 * CPP, w)
            )
        for g in GROUPS:
            wg = W - g
            blk = big[:, XB[g] : XB[g] + G * CPP * wg].reshape(128, G, CPP, wg)
            for i in range(G):
                d = g + i
                w = W - d
                flat[d, r0 + TAIL : r0 + ROWS, d:] = (
                    blk[:, i, :, :w].astype(np.float32).reshape(128 * CPP, w)
                )
        for d in range(MAX_DISP):
            flat[d, r0 : r0 + TAIL, d:] = tail[:, d, : W - d].astype(np.float32)
    vol = flat.reshape(MAX_DISP, N, C, H, W).transpose(1, 2, 0, 3, 4)
    return np.ascontiguousarray(vol)
